# revision 1
# baseline (speedup 1.0000x reference)
"""Self-attention (nn_Attention_85169201480320) as a distributed Bass kernel
on 8 TRN2 NeuronCores.

Reference computation (B=2, S=2048, H=1024, NH=16, HD=64):
    mixed_query = x @ Wq + bq
    query = split_heads(mixed_query @ Wq + bq)     # double-apply bug preserved
    key   = split_heads(x @ Wk + bk)
    value = split_heads(x @ Wv + bv)
    out   = softmax(q k^T / sqrt(HD)) v            # per (batch, head)

Sharding: tensor-parallel over heads — core c owns heads {2c, 2c+1}, i.e.
columns [c*128, (c+1)*128) of the QKV projections and of the output. x is
replicated (pre-transposed and cast to fp16 on host). The double Q
projection is folded on the host: query = x @ (Wq@Wq) + (bq@Wq + bq).

Per-core device graph (no collectives needed):
  - Q^T, K^T in [d, seq] layout: psum = W_chunk^T-stationary @ x^T-moving,
    bias added on VectorE during the PSUM->SBUF copy.
  - V in [seq, d] layout (stationary = x^T chunk, moving = Wv slice), with
    a constant ones column appended (column 64) so the PV matmul also
    computes the softmax row-sums. V bias is deferred to the epilogue
    (softmax rows sum to 1).
  - Attention per (batch b, q-chunk of 512, k-chunk of 128):
      S^T[k, q] for both heads via row-tiled (64-contraction) matmuls into
      one [128, 1024] PSUM tile; one ScalarE Exp (scale=1/8 folded) into an
      fp16 E^T tile; PV matmuls accumulate [65, 512] per head over k-chunks.
  - Epilogue: PE-transpose of [65, 128] C^T tiles -> [128, 65], reciprocal
    of the sumexp column, out = C*recip + bv broadcast, DMA out (SWDGE ring).

Variant 'v5' (champion, 194.7us): v2 emission plan (dense K/Q prefill burst,
consumption-ordered background fills, consolidated DMAs, weights pre-packed
for 2KB DMA lines) + PV pair at PE priority -50k so the S^T->exp chain is
never queued behind it.  Measured dead ends kept for reference: v3 DMA-XBAR
epilogue (231us), v4 minimal prefill (218us), v6-v9 row-split matmuls
(rejected by walrus / slower), v10/v11 buffer+lag tweaks (208/212us).
"""

import numpy as np

B, S, H = 2, 2048, 1024
NH, HD = 16, 64
N_CORES = 8
HPC = NH // N_CORES        # heads per core = 2
CPC = HPC * HD             # output columns per core = 128
BS = B * S                 # 4096 rows total
SCALE = HD ** -0.5
# spare knob: a constant folded out of exp (cancels in the final division
# by the identically-scaled row sums); currently 0 in the exp call below.
EXP_BIAS = 2.25

HC = H // 128              # 8 contraction chunks
SC_ALL = BS // 128         # 32 seq chunks of 128
SC_B = S // 128            # 16 seq chunks per batch
QC_B = S // 512            # 4 q-chunks of 512 per batch
QT_B = S // 128            # 16 q-tiles of 128 per batch


def build_kernel(repeat: int = 1, variant: str = 'bg'):
    import concourse.bass as bass
    import concourse.mybir as mybir
    import concourse.tile as tile
    from concourse import bacc
    from concourse.masks import make_identity

    fp16 = mybir.dt.float16
    f32 = mybir.dt.float32

    nc = bacc.Bacc("TRN2", target_bir_lowering=False, debug=False,
                   num_devices=N_CORES)

    xT = nc.declare_dram_parameter("xT", [H, BS], fp16, isOutput=False)
    # weights pre-packed on host as [128, HC*CPC]: w[p, hc*CPC+m] =
    # W[hc*128+p, m] -> contiguous 2KB DMA lines per partition.
    wq = nc.declare_dram_parameter("wq", [128, HC * CPC], fp16, isOutput=False)
    wk = nc.declare_dram_parameter("wk", [128, HC * CPC], fp16, isOutput=False)
    wv = nc.declare_dram_parameter("wv", [128, HC * CPC], fp16, isOutput=False)
    bq = nc.declare_dram_parameter("bq", [CPC, 1], f32, isOutput=False)
    bk = nc.declare_dram_parameter("bk", [CPC, 1], f32, isOutput=False)
    bv = nc.declare_dram_parameter("bv", [CPC], f32, isOutput=False)
    out = nc.declare_dram_parameter("out", [BS, CPC], f32, isOutput=True)

    with tile.TileContext(nc) as tc:
        with (
            tc.tile_pool(name="big", bufs=1) as big,
            tc.tile_pool(name="work", bufs=2) as work,
            tc.tile_pool(name="psum", bufs=1, space="PSUM") as psum,
        ):
            # ---- constants / small inputs ----
            ident = big.tile([65, 65], f32)
            make_identity(nc, ident)
            # small DMAs ride the SWDGE ring: the SP ring's issue slots are
            # reserved for the weight + xT loads that gate the first matmul.
            bq_sb = big.tile([CPC, 1], f32)
            nc.gpsimd.dma_start(out=bq_sb, in_=bq[:, :])
            bk_sb = big.tile([CPC, 1], f32)
            nc.gpsimd.dma_start(out=bk_sb, in_=bk[:, :])
            # bv broadcast to all 128 partitions: [128, 128]
            bv_sb = big.tile([128, CPC], f32)
            expb_sb = big.tile([128, 1], f32)
            nc.vector.memset(expb_sb, -EXP_BIAS)
            bv_ap = bv.ap()
            bv_bcast = bass.AP(tensor=bv_ap.tensor, offset=bv_ap.offset,
                               ap=[[0, 128], [1, CPC]])
            nc.gpsimd.dma_start(out=bv_sb, in_=bv_bcast)

            # ---- big persistent SBUF tensors ----
            # weights first: the first projection matmuls need w + one xT
            # chunk, so don't queue 8MB of xT DMA ahead of them.
            # Layout [128, 3, HC, CPC] so each weight DMA is one contiguous
            # 2KB/partition line on BOTH sides (host pre-packs).
            w_sb = big.tile([128, 3, HC, CPC], fp16)       # 6KB/part
            for t, w in ((0, wq), (1, wk), (2, wv)):
                nc.sync.dma_start(
                    out=w_sb[:, t, :, :],
                    in_=w.ap().rearrange("p (c m) -> p c m", c=HC))
            xT_sb = big.tile([128, HC, BS], fp16)          # 64KB/part
            # Three large DMAs on the SP ring (NOT the ACT ring: the ACT
            # sequencer must be free to issue the first exp the moment
            # S^T(0) lands).  Each InstDMACopy is split across all 16 SDMA
            # engines, so one big strided DMA moves at full HBM bandwidth;
            # the first covers seq-columns 0:512 of every hc chunk (all
            # K/Q/attention chunk 0 needs), so the exp stream starts early.
            # Columns 0:512 land per-hc (8 small DMAs) so each projection
            # matmul can start the moment its contraction chunk arrives;
            # the rest lands as four large DMAs (each InstDMACopy spreads
            # over all 16 SDMA engines), sliced so the K-projection / exp
            # stream never outruns the data.
            for hc in range(HC):
                nc.sync.dma_start(
                    out=xT_sb[:, hc, 0:512],
                    in_=xT[hc * 128:(hc + 1) * 128, 0:512])
            xT_r = xT.ap().rearrange("(c p) m -> p c m", p=128)
            for lo, hi in ((512, 1024), (1024, 1536), (1536, 2048),
                           (S, BS)):
                nc.sync.dma_start(
                    out=xT_sb[:, :, lo:hi], in_=xT_r[:, :, lo:hi])

            qkT = big.tile([128, 2, BS], fp16)             # 16KB/part
            v_sb = big.tile([128, HPC, SC_ALL, 65], fp16)  # 8.3KB/part
            if variant == 'v3':
                # fp16 C^T staging, 80 partitions (5 XBAR 16-row tiles) so
                # the epilogue transpose runs on the DMA XBAR instead of the
                # PE.  Rows 65:80 are zeroed junk that transposes into
                # columns 65:80 of cT, which the epilogue never reads.
                cuT = big.tile([80, 2 * HPC, S], fp16)     # 16KB/part
                # 64-aligned start partition; row 64 is rewritten by the
                # PV copies before anything reads it.
                nc.vector.memset(cuT[64:80, :, :], 0.0)
            else:
                cuT = big.tile([65, 2 * HPC, S], f32)      # 32KB/part

            # ones column of V_aug (written once; V copies touch only 0:64)
            nc.vector.memset(v_sb[:, :, :, 64:65], 1.0)

            def emit_body():
                _emit_attention_body(nc, tc, bass, mybir, psum, work, big,
                                     xT_sb, w_sb, qkT, v_sb, cuT,
                                     bq_sb, bk_sb, bv_sb, ident, out, expb_sb,
                                     variant)

            if repeat == 1:
                emit_body()
            else:
                with tc.For_i(0, repeat, 1):
                    emit_body()

    nc.finalize()
    return nc


def _emit_attention_body(nc, tc, bass, mybir, psum, work, big,
                         xT_sb, w_sb, qkT, v_sb, cuT,
                         bq_sb, bk_sb, bv_sb, ident, out, expb_sb,
                         variant='bg'):
            fp16 = mybir.dt.float16
            f32 = mybir.dt.float32
            # ---- emission plan ----
            # attention chunk (b, qc) needs: Q chunk sc=4b+qc, ALL of K for
            # batch b, and V chunks racing ahead of its kc loop. So: project
            # K(b0) + Q(b0,sc0) first (hc-outer, so the PE starts on the
            # first 512KB xT DMA), start attention immediately, and feed the
            # remaining Q/K/V projections in as fillers between (and inside)
            # attention chunks, where they soak up PE slack under the
            # ScalarE-paced exp stream.
            def emit_proj_hc_outer(jobs):
                # jobs: list of (t, sc, tag) -> one [128,512] psum tile each
                tiles = [
                    psum.tile([128, 512], f32, tag=tag,
                              bufs=1 if (variant == 'v9' and tag != 'sT')
                              else 2,
                              name=f"pj0_{t}_{sc}")
                    for t, sc, tag in jobs
                ]
                for hc in range(HC):
                    for (t, sc, _), ps in zip(jobs, tiles):
                        if variant == 'v8':
                            for half in (1, 0):
                                r0 = 64 * half
                                nc.tensor.matmul(
                                    ps,
                                    w_sb[r0:r0 + 64, t, hc, :],
                                    xT_sb[r0:r0 + 64, hc,
                                          sc * 512:(sc + 1) * 512],
                                    start=(hc == 0 and half == 1),
                                    stop=(hc == HC - 1 and half == 0),
                                    skip_group_check=True,
                                )
                        else:
                            nc.tensor.matmul(
                                ps,
                                w_sb[:, t, hc, :],
                                xT_sb[:, hc, sc * 512:(sc + 1) * 512],
                                start=(hc == 0), stop=(hc == HC - 1),
                            )
                for (t, sc, _), ps in zip(jobs, tiles):
                    nc.vector.tensor_scalar_add(
                        qkT[:, t, sc * 512:(sc + 1) * 512], ps,
                        bq_sb if t == 0 else bk_sb,
                    )

            def emit_proj(t, sc):
                ps = psum.tile([128, 512], f32, tag="aux",
                               bufs=1 if variant == 'v9' else 2,
                               name=f"pj_{t}_{sc}")
                for hc in range(HC):
                    if variant == 'v8':
                        for half in (1, 0):
                            r0 = 64 * half
                            nc.tensor.matmul(
                                ps,
                                w_sb[r0:r0 + 64, t, hc, :],
                                xT_sb[r0:r0 + 64, hc,
                                      sc * 512:(sc + 1) * 512],
                                start=(hc == 0 and half == 1),
                                stop=(hc == HC - 1 and half == 0),
                                skip_group_check=True,
                            )
                    else:
                        nc.tensor.matmul(
                            ps,
                            w_sb[:, t, hc, :],
                            xT_sb[:, hc, sc * 512:(sc + 1) * 512],
                            start=(hc == 0), stop=(hc == HC - 1),
                        )
                nc.vector.tensor_scalar_add(
                    qkT[:, t, sc * 512:(sc + 1) * 512], ps,
                    bq_sb if t == 0 else bk_sb,
                )

            def emit_v_chunk(sc):
                ps = psum.tile([128, CPC], f32, tag="aux",
                               bufs=1 if variant == 'v9' else 2,
                               name=f"psv_{sc}")
                for hc in range(HC):
                    nc.tensor.matmul(
                        ps,
                        xT_sb[:, hc, sc * 128:(sc + 1) * 128],
                        w_sb[:, 2, hc, :],
                        start=(hc == 0), stop=(hc == HC - 1),
                    )
                # [128, 2, 64] strided copy into v_sb (both heads)
                nc.vector.tensor_copy(
                    v_sb[:, :, sc, 0:64],
                    ps.rearrange("p (h d) -> p h d", h=HPC),
                )

            V = lambda s: (lambda: emit_v_chunk(s))
            P = lambda t, s: (lambda: emit_proj(t, s))

            if variant == 'bgpaced':
                # like 'bg', but gives the scheduler a pacing hint per
                # background piece (earliest useful time, us) so it does not
                # front-stuff V work ahead of the first exp stream.
                emit_proj_hc_outer([(1, 0, "sT"), (1, 1, "sT"),
                                    (1, 2, "pv"), (1, 3, "pv"),
                                    (0, 0, "aux"), (0, 1, "aux")])
                with tc.high_priority(offset=-1_000_000):
                    def at(us, f):
                        with tc.tile_wait_until(us / 1000.0):
                            f()
                    for sc in range(SC_B):
                        at(10 + sc * 1.2, lambda s=sc: emit_v_chunk(s))
                    at(28, lambda: emit_proj(0, 2))
                    at(38, lambda: emit_proj(0, 3))
                    for i, sc in enumerate(range(4, HC)):
                        at(45 + 6 * i, lambda s=sc: emit_proj(1, s))
                    for sc in range(SC_B, SC_ALL):
                        at(55 + (sc - SC_B) * 1.2, lambda s=sc: emit_v_chunk(s))
                    at(72, lambda: emit_proj(0, 4))
                    at(88, lambda: emit_proj(0, 5))
                    at(107, lambda: emit_proj(0, 6))
                    at(126, lambda: emit_proj(0, 7))
                mid_fill = {}
                end_fill = {(b, qc): [] for b in range(B)
                            for qc in range(QC_B)}
            elif variant == 'v12':
                # batch1 prefill only; batch2 + V(0..3) lead the background
                # band -- viable now that PV runs at lower PE priority and
                # lags 2 kc (more slack per kc for fills).
                emit_proj_hc_outer([(1, 0, "sT"), (0, 0, "aux")])
                with tc.high_priority(offset=-1_000_000):
                    emit_proj(1, 1)               # K(0,sc1): kc4
                    emit_v_chunk(0)
                    emit_v_chunk(1)
                    emit_proj(1, 2)               # K(0,sc2): kc8
                    emit_v_chunk(2)
                    emit_v_chunk(3)
                    emit_proj(1, 3)               # K(0,sc3): kc12
                    for sc in range(4, 7):
                        emit_v_chunk(sc)
                    emit_proj(0, 1)               # Q(0,1): chunk (0,1)
                    for sc in range(7, SC_B):
                        emit_v_chunk(sc)
                    emit_proj(0, 2)
                    emit_proj(0, 3)
                    emit_proj(1, 4)
                    emit_proj(0, 4)
                    for sc in range(SC_B, SC_B + 3):
                        emit_v_chunk(sc)
                    emit_proj(1, 5)
                    for sc in range(SC_B + 3, SC_B + 6):
                        emit_v_chunk(sc)
                    emit_proj(1, 6)
                    for sc in range(SC_B + 6, SC_B + 9):
                        emit_v_chunk(sc)
                    emit_proj(1, 7)
                    for sc in range(SC_B + 9, SC_B + 12):
                        emit_v_chunk(sc)
                    emit_proj(0, 5)
                    for sc in range(SC_B + 12, SC_ALL):
                        emit_v_chunk(sc)
                    emit_proj(0, 6)
                    emit_proj(0, 7)
                mid_fill = {}
                end_fill = {(b, qc): [] for b in range(B)
                            for qc in range(QC_B)}
            elif variant == 'v4':
                # minimal prefill: only K(0,sc0)+Q(0,0) gate the first exp;
                # ALL other projections and V chunks go to the background
                # band in consumption order and hide in the kc-loop slack
                # under the ScalarE-paced exp stream.
                emit_proj_hc_outer([(1, 0, "sT"), (0, 0, "aux")])
                with tc.high_priority(offset=-1_000_000):
                    emit_proj(1, 1)               # K(0,sc1): (0,0) kc4
                    for sc in range(0, 3):
                        emit_v_chunk(sc)          # V: PV trails exp 1 kc
                    emit_proj(1, 2)               # K(0,sc2): (0,0) kc8
                    for sc in range(3, 6):
                        emit_v_chunk(sc)
                    emit_proj(1, 3)               # K(0,sc3): (0,0) kc12
                    for sc in range(6, 9):
                        emit_v_chunk(sc)
                    emit_proj(0, 1)               # Q(0,1): chunk (0,1)
                    for sc in range(9, SC_B):
                        emit_v_chunk(sc)
                    emit_proj(0, 2)               # Q(0,2)
                    emit_proj(0, 3)               # Q(0,3)
                    emit_proj(1, 4)               # K(1,sc4): chunk (1,0) kc0
                    emit_proj(0, 4)               # Q(1,0)
                    for sc in range(SC_B, SC_B + 3):
                        emit_v_chunk(sc)
                    emit_proj(1, 5)
                    for sc in range(SC_B + 3, SC_B + 6):
                        emit_v_chunk(sc)
                    emit_proj(1, 6)
                    for sc in range(SC_B + 6, SC_B + 9):
                        emit_v_chunk(sc)
                    emit_proj(1, 7)
                    for sc in range(SC_B + 9, SC_B + 12):
                        emit_v_chunk(sc)
                    emit_proj(0, 5)               # Q(1,1)
                    for sc in range(SC_B + 12, SC_ALL):
                        emit_v_chunk(sc)
                    emit_proj(0, 6)               # Q(1,2)
                    emit_proj(0, 7)               # Q(1,3)
                mid_fill = {}
                end_fill = {(b, qc): [] for b in range(B)
                            for qc in range(QC_B)}
            elif variant in ('v2', 'v3', 'v5', 'v6', 'v7', 'v8', 'v9', 'v10', 'v11', 'v13'):
                # startup: K(b0,sc0)+Q(b0,qc0) first, then rest of K(b0) +
                # Q(b0,qc1); V(0..3) at default band (PV trails exp by one
                # kc); everything else in ONE background band ordered by
                # true consumption time, so the dependency-driven scheduler
                # pulls each piece just-in-time without a later-needed piece
                # blocking an earlier-needed one.
                emit_proj_hc_outer([(1, 0, "sT"), (0, 0, "aux")])
                pvt = ("pv0", "pv1") if variant == 'v9' else ("pv", "pv")
                emit_proj_hc_outer([(1, 1, "sT"), (1, 2, pvt[0]),
                                    (1, 3, pvt[1]), (0, 1, "aux")])
                v_pref = 0 if variant == 'v13' else 4
                for sc in range(v_pref):
                    emit_v_chunk(sc)
                with tc.high_priority(offset=-1_000_000):
                    for sc in range(v_pref, SC_B):
                        emit_v_chunk(sc)          # used from t~5us
                    emit_proj(0, 2)               # Q(0,2): chunk (0,2) start
                    emit_proj(0, 3)               # Q(0,3): chunk (0,3) start
                    emit_proj(1, 4)               # K(1,sc4): chunk (1,0) kc0
                    emit_proj(0, 4)               # Q(1,0): chunk (1,0) start
                    for sc in range(SC_B, SC_B + 3):
                        emit_v_chunk(sc)          # V(b1): kc-paced
                    emit_proj(1, 5)               # K(1,sc5): (1,0) kc4
                    for sc in range(SC_B + 3, SC_B + 6):
                        emit_v_chunk(sc)
                    emit_proj(1, 6)               # K(1,sc6): (1,0) kc8
                    for sc in range(SC_B + 6, SC_B + 9):
                        emit_v_chunk(sc)
                    emit_proj(1, 7)               # K(1,sc7): (1,0) kc12
                    for sc in range(SC_B + 9, SC_B + 12):
                        emit_v_chunk(sc)
                    emit_proj(0, 5)               # Q(1,1): chunk (1,1) start
                    for sc in range(SC_B + 12, SC_ALL):
                        emit_v_chunk(sc)
                    emit_proj(0, 6)               # Q(1,2): chunk (1,2) start
                    emit_proj(0, 7)               # Q(1,3): chunk (1,3) start
                mid_fill = {}
                end_fill = {(b, qc): [] for b in range(B)
                            for qc in range(QC_B)}
            elif variant == 'bg':
                # K(b0) + the first two Q chunks up front (hc-outer so the
                # PE tracks the xT DMA); everything else -- remaining Q/K
                # projections and all V chunks -- is emitted ONCE at
                # background priority, in rough consumption order. The Tile
                # scheduler then runs it in PE idle slots, and data
                # dependencies pull each piece in just-in-time.
                emit_proj_hc_outer([(1, 0, "sT"), (0, 0, "aux")])
                pvt = ("pv0", "pv1") if variant == 'v9' else ("pv", "pv")
                emit_proj_hc_outer([(1, 1, "sT"), (1, 2, pvt[0]),
                                    (1, 3, pvt[1]), (0, 1, "aux")])
                for sc in range(8):
                    emit_v_chunk(sc)
                with tc.high_priority(offset=-1_000_000):
                    for sc in range(8, SC_B):
                        emit_v_chunk(sc)
                    for sc in range(4, HC):
                        emit_proj(1, sc)
                    emit_proj(0, 2)
                    emit_proj(0, 3)
                    for sc in range(SC_B, SC_ALL):
                        emit_v_chunk(sc)
                    for sc in range(4, HC):
                        emit_proj(0, sc)
                mid_fill = {}
                end_fill = {(b, qc): [] for b in range(B)
                            for qc in range(QC_B)}
            elif variant == 'midfill':
                # startup: K(b0) fully (every attention chunk of b0 needs
                # all of K), Q chunk 0, and the first V chunks. Everything
                # else fills PE slack inside attention chunks via mid_fill:
                # mid_fill[(b,qc)][kc] = thunks after that kc iteration,
                # paced ~1 V chunk (or 1/2 proj tile) per iteration, with a
                # >=3-iteration lead on the consuming PV.
                emit_proj_hc_outer([(1, 0, "sT"), (1, 1, "sT"),
                                    (1, 2, "pv"), (1, 3, "pv"),
                                    (0, 0, "aux")])
                for sc in range(4):
                    emit_v_chunk(sc)
                mid_fill = {
                    (0, 0): {**{kc: [V(3 + kc)] for kc in range(1, 13)},
                             13: [P(0, 1)]},
                    (0, 1): {2: [P(1, 4)], 7: [P(0, 2)], 12: [P(1, 5)]},
                    (0, 2): {2: [P(1, 6)], 7: [P(0, 3)], 12: [P(1, 7)]},
                    (0, 3): {**{kc: [V(14 + kc)] for kc in range(2, 10)},
                             11: [P(0, 4)]},
                    (1, 0): {**{kc: [V(23 + kc)] for kc in range(1, 9)}},
                }
                end_fill = {
                    (0, 0): [], (0, 1): [], (0, 2): [], (0, 3): [],
                    (1, 0): [P(0, 5)], (1, 1): [P(0, 6)],
                    (1, 2): [P(0, 7)], (1, 3): [],
                }
            else:  # 'upfront'
                emit_proj_hc_outer([(1, 0, "sT"), (1, 1, "sT"),
                                    (1, 2, "pv"), (1, 3, "pv"),
                                    (0, 0, "aux"), (0, 1, "aux")])
                for sc in range(SC_B):
                    emit_v_chunk(sc)

                def b1_slice(i):
                    t = i % 2
                    sc = 4 + 2 * (i // 2)
                    for s in (sc, sc + 1):
                        emit_proj(t, s)
                    for s in range(SC_B + 4 * i, SC_B + 4 * i + 4):
                        emit_v_chunk(s)

                mid_fill = {}
                end_fill = {
                    (0, 0): [P(0, 2), lambda: b1_slice(0)],
                    (0, 1): [P(0, 3), lambda: b1_slice(1)],
                    (0, 2): [lambda: b1_slice(2)],
                    (0, 3): [lambda: b1_slice(3)],
                    (1, 0): [], (1, 1): [], (1, 2): [], (1, 3): [],
                }

            # ---- attention ----
            for b in range(B):
                for qc in range(QC_B):  # q-chunks of 512
                    q0 = b * S + qc * 512
                    if variant == 'v9':
                        pv = [
                            psum.tile([65, 512], f32, tag=f"pv{i}", bufs=1,
                                      name=f"pv_{b}_{qc}_{i}")
                            for i in range(3)
                        ]  # 0: h0 rows 0-63, 1: h0 rows 64-127, 2: h1
                    else:
                        pv = [
                            psum.tile([65, 512], f32, tag="pv", bufs=2,
                                      name=f"pv_{b}_{qc}_{h}")
                            for h in range(HPC)
                        ]
                    # kc loop, software-pipelined: PV trails one iteration
                    # so the PE always issues the next S^T (which feeds the
                    # ScalarE exp stream, the pacer) before the current PV.
                    eTs = {}

                    def emit_pv(kc):
                        eT_prev = eTs.pop(kc)
                        ctx = tc.high_priority(offset=-50_000) \
                            if variant in ('v5', 'v6', 'v7', 'v8', 'v9',
                                           'v10', 'v11', 'v12', 'v13') else None
                        if ctx is not None:
                            ctx.__enter__()
                        if variant == 'v9':
                            last = (kc == SC_B - 1)
                            for half in (1, 0):
                                r0 = 64 * half
                                nc.tensor.matmul(
                                    pv[half],
                                    v_sb[r0:r0 + 64, 0, b * SC_B + kc, :],
                                    eT_prev[r0:r0 + 64, 0:512],
                                    start=(kc == 0), stop=last,
                                )
                            nc.tensor.matmul(
                                pv[2],
                                v_sb[:, 1, b * SC_B + kc, :],
                                eT_prev[:, 512:1024],
                                start=(kc == 0), stop=last,
                            )
                        elif variant in ('v6', 'v7', 'v8') and kc > 0:
                            # Split each head's PV into two 64-contraction
                            # halves on disjoint PE row groups -> the pair
                            # co-issues in the array (row tiling).  Both
                            # halves accumulate into the same PSUM elements;
                            # adds are commutative and kc=0 already set the
                            # has_written bits, so drain order is harmless.
                            last = (kc == SC_B - 1)
                            for h in range(HPC):
                                for half in (1, 0):
                                    r0 = 64 * half
                                    nc.tensor.matmul(
                                        pv[h],
                                        v_sb[r0:r0 + 64, h, b * SC_B + kc, :],
                                        eT_prev[r0:r0 + 64,
                                                h * 512:(h + 1) * 512],
                                        start=False,
                                        stop=(last and half == 0),
                                        skip_group_check=True,
                                    )
                        else:
                            for h in range(HPC):
                                nc.tensor.matmul(
                                    pv[h],
                                    v_sb[:, h, b * SC_B + kc, :],
                                    eT_prev[:, h * 512:(h + 1) * 512],
                                    start=(kc == 0),
                                    stop=(kc == SC_B - 1 and
                                          variant not in ('v6', 'v7', 'v8')),
                                )
                        if ctx is not None:
                            ctx.__exit__(None, None, None)

                    for kc in range(SC_B):  # k-chunks of 128
                        k0 = b * S + kc * 128
                        sT = psum.tile([128, 1024], f32, tag="sT", bufs=2,
                                       name=f"sT_{b}_{qc}_{kc}")
                        if variant in ('v7', 'v8'):
                            # 4 concurrent 32-contraction tiles (one per PE
                            # row group).  Per head, the half nearer the
                            # drain edge carries start=True (clears) and is
                            # emitted first; the other half accumulates.
                            for h in range(HPC):
                                for half in (1, 0):
                                    r0 = h * 64 + 32 * half
                                    nc.tensor.matmul(
                                        sT[:, h * 512:(h + 1) * 512],
                                        qkT[r0:r0 + 32, 1, k0:k0 + 128],
                                        qkT[r0:r0 + 32, 0, q0:q0 + 512],
                                        start=(half == 1), stop=(half == 0),
                                        skip_group_check=True,
                                        tile_position=(r0, 0),
                                    )
                        else:
                            for h in range(HPC):
                                r0, r1 = h * 64, (h + 1) * 64
                                nc.tensor.matmul(
                                    sT[:, h * 512:(h + 1) * 512],
                                    qkT[r0:r1, 1, k0:k0 + 128],
                                    qkT[r0:r1, 0, q0:q0 + 512],
                                    start=True, stop=True,
                                )
                        eT = work.tile([128, 1024], fp16, tag="eT",
                                       bufs=4 if variant in ('v10', 'v11', 'v12')
                                       else 3,
                                       name=f"eT_{b}_{qc}_{kc}")
                        if variant == 'v13':
                            # one exp per head: each waits only its own S^T
                            # matmul, so eT halves land earlier and the sT
                            # buffer frees sooner for S^T(kc+2).
                            for h in range(HPC):
                                nc.scalar.activation(
                                    eT[:, h * 512:(h + 1) * 512],
                                    sT[:, h * 512:(h + 1) * 512],
                                    mybir.ActivationFunctionType.Exp,
                                    bias=0.0, scale=SCALE,
                                )
                        else:
                            nc.scalar.activation(
                                eT, sT, mybir.ActivationFunctionType.Exp,
                                bias=0.0, scale=SCALE,
                            )
                        eTs[kc] = eT
                        pv_lag = 2 if variant in ('v11', 'v12') else 1
                        if kc >= pv_lag:
                            emit_pv(kc - pv_lag)
                        for f in mid_fill.get((b, qc), {}).get(kc, ()):
                            f()
                    for kc_t in range(SC_B - pv_lag, SC_B):
                        emit_pv(kc_t)
                    if variant == 'v9':
                        cslice = cuT[0:65, 2 * b, qc * 512:(qc + 1) * 512]
                        nc.vector.tensor_copy(cslice, pv[1])
                        nc.vector.tensor_tensor(
                            out=cslice, in0=pv[0], in1=cslice,
                            op=mybir.AluOpType.add)
                        nc.vector.tensor_copy(
                            cuT[0:65, 2 * b + 1, qc * 512:(qc + 1) * 512],
                            pv[2])
                    else:
                        for h in range(HPC):
                            nc.vector.tensor_copy(
                                cuT[0:65, 2 * b + h, qc * 512:(qc + 1) * 512],
                                pv[h]
                            )

                    # epilogue for this chunk: transpose, normalize, bias,
                    # store. Mid-band priority: deferred into PE idle slots,
                    # but ahead of the projection/V background.
                    ep_off = -800_000 if variant == 'v10' else -500_000
                    ep_ctx = tc.high_priority(offset=ep_off) \
                        if variant not in ('upfront', 'midfill') else None
                    if ep_ctx is not None:
                        ep_ctx.__enter__()
                    for qt in range(qc * 4, qc * 4 + 4):  # q-tiles of 128
                        o_sb = work.tile([128, CPC], f32, tag="osb", bufs=3,
                                         name=f"osb_{b}_{qt}")
                        for h in range(HPC):
                            if variant == 'v3':
                                # DMA-XBAR transpose (frees the PE): [80,128]
                                # fp16 -> [128,80]; cols 64=sumexp, 65:80 junk
                                cT = work.tile([128, 80], fp16, tag="ct",
                                               bufs=4, name=f"ct_{b}_{qt}_{h}")
                                nc.sync.dma_start_transpose(
                                    out=cT,
                                    in_=cuT[:, 2 * b + h,
                                            qt * 128:(qt + 1) * 128])
                                tr = cT
                            else:
                                tr = psum.tile([128, 65], f32, tag="aux",
                                               bufs=1 if variant == 'v9'
                                               else 2,
                                               name=f"tr_{b}_{qt}_{h}")
                                nc.tensor.transpose(
                                    tr,
                                    cuT[:, 2 * b + h, qt * 128:(qt + 1) * 128],
                                    ident,
                                )
                            rec = work.tile([128, 1], f32, tag="rec", bufs=4,
                                            name=f"rec_{b}_{qt}_{h}")
                            nc.vector.reciprocal(rec, tr[:, 64:65])
                            nc.vector.scalar_tensor_tensor(
                                o_sb[:, h * 64:(h + 1) * 64],
                                tr[:, 0:64], rec, bv_sb[:, h * 64:(h + 1) * 64],
                                op0=mybir.AluOpType.mult,
                                op1=mybir.AluOpType.add,
                            )
                        r0 = b * S + qt * 128
                        # out DMAs ride the SWDGE (gpsimd) ring so they never
                        # queue behind the multi-MB xT loads on the SP ring.
                        dma_eng = nc.sync if variant in ('bg', 'bgpaced', 'upfront', 'midfill') else nc.gpsimd
                        dma_eng.dma_start(out=out[r0:r0 + 128, :], in_=o_sb)
                    if ep_ctx is not None:
                        ep_ctx.__exit__(None, None, None)
                    for f in end_fill[(b, qc)]:
                        f()


def prep_inputs(x, Wq, bq, Wk, bk, Wv, bv):
    """Host-side prep: fold the double Q projection, transpose/cast x,
    slice per-core weights."""
    x = np.asarray(x, np.float32)
    Wq = np.asarray(Wq, np.float64)
    bq = np.asarray(bq, np.float64)
    Wq2 = (Wq @ Wq).astype(np.float32)
    bq2 = (bq @ Wq + bq).astype(np.float32)
    Wk = np.asarray(Wk, np.float32)
    Wv = np.asarray(Wv, np.float32)
    bk = np.asarray(bk, np.float32)
    bv = np.asarray(bv, np.float32)

    xT = np.ascontiguousarray(x.reshape(BS, H).T).astype(np.float16)

    def pack_w(w_slice):
        # [H, CPC] -> [128, HC*CPC] with w[p, hc*CPC+m] = W[hc*128+p, m]
        return np.ascontiguousarray(
            w_slice.reshape(HC, 128, CPC).transpose(1, 0, 2)
            .reshape(128, HC * CPC)).astype(np.float16)

    in_maps = []
    for c in range(N_CORES):
        lo, hi = c * CPC, (c + 1) * CPC
        in_maps.append({
            "xT": xT,
            "wq": pack_w(Wq2[:, lo:hi]),
            "wk": pack_w(Wk[:, lo:hi]),
            "wv": pack_w(Wv[:, lo:hi]),
            "bq": np.ascontiguousarray(bq2[lo:hi]).reshape(CPC, 1),
            "bk": np.ascontiguousarray(bk[lo:hi]).reshape(CPC, 1),
            "bv": np.ascontiguousarray(bv[lo:hi]),
        })
    return in_maps


_CACHED = {}
DEFAULT_VARIANT = 'v5'


def kernel(x, Wq, bq, Wk, bk, Wv, bv):
    from concourse.bass_utils import run_bass_kernel_spmd

    if "nc" not in _CACHED:
        _CACHED["nc"] = build_kernel(variant=DEFAULT_VARIANT)
    nc = _CACHED["nc"]

    in_maps = prep_inputs(x, Wq, bq, Wk, bk, Wv, bv)
    res = run_bass_kernel_spmd(nc, in_maps, core_ids=list(range(N_CORES)))

    full = np.empty((BS, NH * HD), np.float32)
    for c in range(N_CORES):
        full[:, c * CPC:(c + 1) * CPC] = res.results[c]["out"]
    return full.reshape(B, S, NH * HD)


if __name__ == "__main__":
    nc = build_kernel()
    print("built ok")

